# revision 33
# baseline (speedup 1.0000x reference)
"""3-layer dense GAT (N=4096, F=512, H=8 heads, D=64) on 8 TRN2 NeuronCores.

Strategy (1D row-parallel, v3):
  - Each core owns LOCAL=512 query rows. Per layer Phase A computes
    hext = x_local @ Wext with 4-head-wide matmuls (FD=272), then per head
    AllGathers [h'=exp(f2)*h | ec=exp(f2) | f2a_hi | f2a_lo | r=exp((a-1)f2)]
    (8 small collectives per layer, launched as heads complete).
  - Attention P^T tile [j=128, i=512] = M * max(ea_i, vr_ij), with exp(f2_j)
    folded into the gathered stationary (h' cols; ec col = softmax denom Z).
      vr = exp(alpha*f1_i + lnr_j)     -- SCALAR act w/ per-partition bias
           (lnr = f2a_hi + f2a_lo, exact (a-1)*f2 to ~1e-4), or
      vr = r_j * eb_i                  -- GPSIMD tensor_scalar (some tiles)
      tm = max(vr2, ea2)               -- VECTOR tt paired over 2 heads
      pm = tm * mask                   -- VECTOR / GPSIMD tt per head
    All DVE ops are pure tensor_tensor (2x DVE mode); per-partition-scalar
    work rides the scalar engine (bias AP) and gpsimd.
  - Matmul: out[d,i] += [h'|ec].T @ pm over 32 j-chunks; row 64 = Z.
    h' = U/Z then ELU lands in [d,i] orientation = next layer's lhsT.
"""

import numpy as np
import ml_dtypes

import concourse.bass as bass
import concourse.mybir as mybir
from concourse import bacc, tile, masks
from concourse.bass_utils import run_bass_kernel_spmd

N = 4096
F = 512
D = 64
H = 8
NCORES = 8
LOCAL = N // NCORES          # 512 query rows per core
JC = N // 128                # 32 j-chunks
IC = LOCAL // 128            # 4 local i-chunks
FC = F // 128                # 4 contraction chunks
NL = 3
SLOT = 68                    # 64 h' + ec + f2a_hi + f2a_lo + r
CH = H * SLOT                # 544: per-j-chunk stride in GS
HSLOT = 4 * SLOT             # 272: half-phase-A matmul width
ALPHA = 0.2
f32 = mybir.dt.float32
bf16 = mybir.dt.bfloat16
BF = ml_dtypes.bfloat16
OP = mybir.AluOpType
AF = mybir.ActivationFunctionType


def mask_on_g(jc: int) -> bool:
    # ~14/32 mask multiplies per head on gpsimd
    return jc % 5 < 2 or jc == 30


# C-phase head groups: pairs share one wide max op.
CGROUPS = [[0, 1], [2, 3], [4, 5], [6, 7]]


def build_nc():
    nc = bacc.Bacc(None, target_bir_lowering=False, num_devices=NCORES)

    xT_d = nc.dram_tensor("xT", [F, LOCAL], bf16, kind="ExternalInput")
    maskT_d = nc.dram_tensor("maskT", [N, LOCAL], bf16, kind="ExternalInput")
    wext_d = nc.dram_tensor("wext", [NL, F, H * SLOT], bf16, kind="ExternalInput")
    out_d = nc.dram_tensor("outT", [D, LOCAL], f32, kind="ExternalOutput")

    with tile.TileContext(nc) as tc:
        with (
            tc.tile_pool(name="persist", bufs=1) as pp,
            tc.tile_pool(name="ident", bufs=1) as ident_pool,
            tc.tile_pool(name="small", bufs=3) as sm_pool,
            tc.tile_pool(name="hcz", bufs=2) as hcz_pool,
            tc.tile_pool(name="lnr", bufs=4) as lnr_pool,
            tc.tile_pool(name="vr", bufs=6) as vr_pool,
            tc.tile_pool(name="tm", bufs=6) as tm_pool,
            tc.tile_pool(name="pm", bufs=6) as pm_pool,
            tc.tile_pool(name="norm", bufs=2) as nm_pool,
            tc.tile_pool(name="psA", bufs=3, space="PSUM") as psA,
            tc.tile_pool(name="psB", bufs=3, space="PSUM") as psB,
            tc.tile_pool(name="psT", bufs=2, space="PSUM") as psT,
            tc.tile_pool(name="dram", bufs=1, space="DRAM") as dram,
        ):
            # ---- persistent SBUF ----
            XT = pp.tile([128, FC * LOCAL], bf16, tag="XT")        # x^T local
            MASK = pp.tile([128, JC * LOCAL], bf16, tag="MASK")    # mask^T
            WEXT = pp.tile([128, NL * FC * H * SLOT], bf16, tag="WEXT")
            GS = pp.tile([128, JC * CH], bf16, tag="GS")           # gathered stationary
            F1B = pp.tile([128, H * LOCAL], f32, tag="F1B")        # f1 row bcast/head
            EAT = pp.tile([128, H * LOCAL], bf16, tag="EAT")       # exp(f1)
            F12 = pp.tile([128, IC * H * 2], f32, tag="F12")       # f1,f2 stash
            ACCa = pp.tile([D, LOCAL], f32, tag="ACCa")
            ACCb = pp.tile([D, LOCAL], f32, tag="ACCb")
            OUTS = pp.tile([D, LOCAL], f32, tag="OUTS")
            IDENT = ident_pool.tile([128, 128], f32)

            # DRAM bounce buffers
            LGs = [
                dram.tile([LOCAL, SLOT], bf16, tag=f"LG{h}", name=f"LG{h}")
                for h in range(H)
            ]
            GGs = [
                [
                    dram.tile(
                        [N, SLOT], bf16, tag=f"GG{l}_{h}", addr_space="Shared",
                        name=f"GG{l}_{h}",
                    )
                    for h in range(H)
                ]
                for l in range(NL)
            ]
            EDRF = dram.tile([H, LOCAL], f32, tag="EDRF")     # f1 rows bounce
            RBD = dram.tile([1, LOCAL], f32, tag="RBD")       # 1/Z bounce

            # views
            X2 = XT[:].rearrange("p (fc i) -> p fc i", i=LOCAL)
            M2 = MASK[:].rearrange("p (c i) -> p c i", i=LOCAL)
            W4 = WEXT[:].rearrange("p (l fc s) -> p l fc s", l=NL, fc=FC)
            GS3 = GS[:].rearrange("p (h c s) -> p h c s", h=H, s=SLOT)
            F1B2 = F1B[:].rearrange("p (h i) -> p h i", h=H)
            F12v = F12[:].rearrange("p (ic h t) -> p ic h t", h=H, t=2)
            EDRF2 = EDRF[:].rearrange("h (ic c) -> h ic c", c=128)

            # ---- one-time loads ----
            nc.sync.dma_start(X2, xT_d[:].rearrange("(fc p) i -> p fc i", p=128))
            nc.sync.dma_start(M2, maskT_d[:].rearrange("(c p) i -> p c i", p=128))
            nc.sync.dma_start(
                W4, wext_d[:].rearrange("l (fc p) s -> p l fc s", p=128)
            )
            masks.make_identity(nc, IDENT[:])

            for l in range(NL):
                # ---- Phase A: hext = x @ Wext, 4 heads per matmul ----
                for half in range(2):
                    hvs = []
                    for hh in range(4):
                        hcz = hcz_pool.tile(
                            [128, IC * SLOT], bf16, tag=f"hcz{hh}",
                            name=f"hcz{hh}",
                        )
                        hvs.append(
                            hcz[:].rearrange("p (ic s) -> p ic s", s=SLOT)
                        )
                    for ic in range(IC):
                        ps = psA.tile([128, HSLOT], f32, tag="psA")
                        for fc in range(FC):
                            nc.tensor.matmul(
                                ps[:],
                                X2[:, fc, ic * 128 : (ic + 1) * 128],
                                W4[:, l, fc, half * HSLOT : (half + 1) * HSLOT],
                                start=(fc == 0),
                                stop=(fc == FC - 1),
                            )
                        psv = ps[:].rearrange("p (hh s) -> p hh s", s=SLOT)
                        # stash f1,f2 + ec-scaled h' per head chunk
                        ec4 = sm_pool.tile([128, 4], f32, tag="ec4")
                        nc.scalar.activation(ec4[:], psv[:, :, D + 1], AF.Exp)
                        for hh in range(4):
                            h = half * 4 + hh
                            nc.vector.tensor_copy(
                                F12v[:, ic, h, :], psv[:, hh, D : D + 2]
                            )
                            nc.scalar.activation(
                                hvs[hh][:, ic, 0:D], psv[:, hh, 0:D],
                                AF.Copy, scale=ec4[:, hh : hh + 1],
                            )
                    # per-head tail for this half: gather cols + A2 + collective
                    for hh in range(4):
                        h = half * 4 + hh
                        hv = hvs[hh]
                        # ec column (bf16)
                        nc.scalar.activation(
                            hv[:, :, D], F12v[:, :, h, 1], AF.Exp
                        )
                        # f2a = (a-1)*f2 split hi/lo (exact to ~1e-4)
                        fa32 = sm_pool.tile([128, IC], f32, tag="fa32")
                        nc.gpsimd.tensor_scalar(
                            fa32[:], F12v[:, :, h, 1], ALPHA - 1.0, None, OP.mult
                        )
                        nc.gpsimd.tensor_copy(hv[:, :, D + 1], fa32[:])
                        nc.gpsimd.tensor_tensor(
                            hv[:, :, D + 2], fa32[:], hv[:, :, D + 1],
                            OP.subtract,
                        )
                        nc.scalar.dma_start(
                            LGs[h][:].rearrange("(ic p) s -> p ic s", p=128)[
                                :, :, 0 : D + 3
                            ],
                            hv[:, :, 0 : D + 3],
                        )
                        # A2: f1 row -> broadcast f1 (f32), exp rows (bf16)
                        pt = psT.tile([IC, 128], f32, tag="psT")
                        nc.tensor.transpose(pt[:], F12v[:, :, h, 0], IDENT[:])
                        f1r = sm_pool.tile([IC, 128], f32, tag="f1r")
                        nc.vector.tensor_copy(f1r[:], pt[:])
                        nc.sync.dma_start(EDRF2[h], f1r[:])
                        nc.sync.dma_start(
                            F1B2[:, h, :],
                            EDRF[h : h + 1, :].partition_broadcast(128),
                        )
                        # exp(f1) broadcast derived on-chip from F1B
                        nc.scalar.activation(
                            EAT[:, h * LOCAL : (h + 1) * LOCAL],
                            F1B2[:, h, :], AF.Exp,
                        )
                        nc.gpsimd.collective_compute(
                            "AllGather",
                            OP.bypass,
                            replica_groups=[list(range(NCORES))],
                            ins=[LGs[h].opt()],
                            outs=[GGs[l][h].opt()],
                        )

                # ---- Phase B: prefetch all staging (sync queue) ----
                for h in range(H):
                    gsrc = GGs[l][h][:].rearrange("(c p) s -> p c s", p=128)
                    nc.sync.dma_start(GS3[:, h, :, :], gsrc[:, :, :])

                # ---- Phase C/D per head group ----
                for grp in CGROUPS:
                    ng = len(grp)
                    h0 = grp[0]
                    lnrs = []
                    for h in grp:
                        lnr = lnr_pool.tile(
                            [128, JC], f32, tag=f"lnr{h % 2}", name="lnr"
                        )
                        nc.gpsimd.tensor_tensor(
                            lnr[:],
                            GS3[:, h, :, D + 1],
                            GS3[:, h, :, D + 2],
                            OP.add,
                        )
                        lnrs.append(lnr)

                    pbs = []
                    for k in range(ng):
                        pb = psB.tile(
                            [SLOT - 3, LOCAL], f32, tag="psB", name=f"pb{k}"
                        )
                        pbs.append(pb)
                    for jc in range(JC):
                        vr2 = vr_pool.tile(
                            [128, ng * LOCAL], bf16, tag=f"vr2_{ng}"
                        )
                        for k, h in enumerate(grp):
                            dst = vr2[:, k * LOCAL : (k + 1) * LOCAL]
                            nc.scalar.activation(
                                dst, F1B2[:, h, :], AF.Exp,
                                bias=lnrs[k][:, jc : jc + 1], scale=ALPHA,
                            )
                        tm2 = tm_pool.tile(
                            [128, ng * LOCAL], bf16, tag=f"tm2_{ng}"
                        )
                        nc.vector.tensor_tensor(
                            tm2[:], vr2[:],
                            EAT[:, h0 * LOCAL : (h0 + ng) * LOCAL], OP.max,
                        )
                        for k, h in enumerate(grp):
                            pm = pm_pool.tile([128, LOCAL], bf16, tag="pm")
                            eng = nc.gpsimd if mask_on_g(jc) else nc.vector
                            eng.tensor_tensor(
                                pm[:], tm2[:, k * LOCAL : (k + 1) * LOCAL],
                                M2[:, jc, :], OP.mult,
                            )
                            nc.tensor.matmul(
                                pbs[k][:],
                                GS3[:, h, jc, 0 : D + 1],
                                pm[:],
                                start=(jc == 0),
                                stop=(jc == JC - 1),
                            )

                    # ---- Phase D per head: h' = U/Z, ELU ----
                    for k, h in enumerate(grp):
                        pb = pbs[k]
                        zrow1 = nm_pool.tile([1, LOCAL], f32, tag="zrow1")
                        nc.vector.tensor_copy(zrow1[:], pb[D : D + 1, :])
                        zz4 = nm_pool.tile([128, LOCAL // 128], f32, tag="zz4")
                        nc.sync.dma_start(zz4[:], zrow1[:])
                        rz4 = nm_pool.tile([128, LOCAL // 128], f32, tag="rz4")
                        nc.vector.reciprocal(rz4[:], zz4[:])
                        nc.sync.dma_start(RBD[:], rz4[:])
                        rb = nm_pool.tile([D, LOCAL], f32, tag="rb")
                        nc.sync.dma_start(
                            rb[:], RBD[0:1, :].partition_broadcast(D)
                        )
                        y = nm_pool.tile([D, LOCAL], f32, tag="y")
                        nc.vector.tensor_tensor(y[:], pb[0:D, :], rb[:], OP.mult)
                        ee = nm_pool.tile([D, LOCAL], f32, tag="ee")
                        nc.scalar.activation(ee[:], y[:], AF.Exp)
                        ry = nm_pool.tile([D, LOCAL], f32, tag="ry")
                        nc.scalar.activation(ry[:], y[:], AF.Relu)
                        z1 = nm_pool.tile([D, LOCAL], f32, tag="z1")
                        nc.vector.scalar_tensor_tensor(
                            z1[:], ee[:], 1.0, ry[:], OP.min, OP.add
                        )
                        if l < NL - 1:
                            poff = (h % 2) * D
                            dst = X2[poff : poff + D, h // 2, :]
                            nc.vector.tensor_scalar(
                                dst, z1[:], 1.0, None, OP.subtract
                            )
                        else:
                            ey = nm_pool.tile([D, LOCAL], f32, tag="ey")
                            nc.vector.tensor_scalar(
                                ey[:], z1[:], 1.0, None, OP.subtract
                            )
                            if h == 0:
                                nc.vector.tensor_copy(ACCa[:], ey[:])
                            else:
                                src, dst_acc = (
                                    (ACCa, ACCb) if h % 2 == 1 else (ACCb, ACCa)
                                )
                                nc.vector.tensor_tensor(
                                    dst_acc[:], src[:], ey[:], OP.add
                                )

            # ---- final: mean over heads, ELU, write out ----
            fin = ACCb if (H - 1) % 2 == 1 else ACCa
            m1 = nm_pool.tile([D, LOCAL], f32, tag="m1")
            nc.vector.tensor_scalar(m1[:], fin[:], 1.0 / H, None, OP.mult)
            e2 = nm_pool.tile([D, LOCAL], f32, tag="e2")
            nc.scalar.activation(e2[:], m1[:], AF.Exp)
            r2 = nm_pool.tile([D, LOCAL], f32, tag="r2")
            nc.scalar.activation(r2[:], m1[:], AF.Relu)
            nc.vector.scalar_tensor_tensor(
                OUTS[:], e2[:], 1.0, r2[:], OP.min, OP.add
            )
            nc.vector.tensor_scalar(OUTS[:], OUTS[:], 1.0, None, OP.subtract)
            nc.sync.dma_start(out_d[:], OUTS[:])

    nc.compile()
    return nc


def _prep_inputs(inputs):
    x = np.asarray(inputs["x"], np.float32)
    adj = np.asarray(inputs["adj"])
    Ws = [np.asarray(inputs[k], np.float32) for k in ("W1", "W2", "W3")]
    a1s = [np.asarray(inputs[k], np.float32) for k in ("a1_1", "a1_2", "a1_3")]
    a2s = [np.asarray(inputs[k], np.float32) for k in ("a2_1", "a2_2", "a2_3")]

    wext = np.zeros((NL, F, H * SLOT), np.float32)
    for l in range(NL):
        for h in range(H):
            wext[l, :, h * SLOT : h * SLOT + D] = Ws[l][h]
            wext[l, :, h * SLOT + D] = Ws[l][h] @ a1s[l][h]
            wext[l, :, h * SLOT + D + 1] = Ws[l][h] @ a2s[l][h]
    wext_bf = np.ascontiguousarray(wext.astype(BF))

    mask = adj > 0
    in_maps = []
    for c in range(NCORES):
        rows = slice(c * LOCAL, (c + 1) * LOCAL)
        in_maps.append(
            {
                "xT": np.ascontiguousarray(x[rows].T).astype(BF),
                "maskT": np.ascontiguousarray(mask[rows].T).astype(BF),
                "wext": wext_bf,
            }
        )
    return in_maps


_CACHE = {}


def _run(inputs, trace=False):
    in_maps = _prep_inputs(inputs)
    if "nc" not in _CACHE:
        _CACHE["nc"] = build_nc()
    res = run_bass_kernel_spmd(
        _CACHE["nc"], in_maps, list(range(NCORES)), trace=trace
    )
    outs = [r["outT"] for r in res.results]
    out = np.concatenate([np.asarray(o, np.float32).T for o in outs], axis=0)
    return out, res


def kernel(**inputs) -> np.ndarray:
    out, _ = _run(inputs, trace=False)
    return out.astype(np.float32)


# revision 35
# speedup vs baseline: 1.1113x; 1.1113x over previous
"""3-layer dense GAT (N=4096, F=512, H=8 heads, D=64) on 8 TRN2 NeuronCores.

Strategy (1D row-parallel, v3):
  - Each core owns LOCAL=512 query rows. Per layer Phase A computes
    hext = x_local @ Wext with 4-head-wide matmuls (FD=272), then per head
    AllGathers [h'=exp(f2)*h | ec=exp(f2) | f2a_hi | f2a_lo | r=exp((a-1)f2)]
    (8 small collectives per layer, launched as heads complete).
  - Attention P^T tile [j=128, i=512] = M * max(ea_i, vr_ij), with exp(f2_j)
    folded into the gathered stationary (h' cols; ec col = softmax denom Z).
      vr = exp(alpha*f1_i + lnr_j)     -- SCALAR act w/ per-partition bias
           (lnr = f2a_hi + f2a_lo, exact (a-1)*f2 to ~1e-4), or
      vr = r_j * eb_i                  -- GPSIMD tensor_scalar (some tiles)
      tm = max(vr2, ea2)               -- VECTOR tt paired over 2 heads
      pm = tm * mask                   -- VECTOR / GPSIMD tt per head
    All DVE ops are pure tensor_tensor (2x DVE mode); per-partition-scalar
    work rides the scalar engine (bias AP) and gpsimd.
  - Matmul: out[d,i] += [h'|ec].T @ pm over 32 j-chunks; row 64 = Z.
    h' = U/Z then ELU lands in [d,i] orientation = next layer's lhsT.
"""

import numpy as np
import ml_dtypes

import concourse.bass as bass
import concourse.mybir as mybir
from concourse import bacc, tile, masks
from concourse.bass_utils import run_bass_kernel_spmd

N = 4096
F = 512
D = 64
H = 8
NCORES = 8
LOCAL = N // NCORES          # 512 query rows per core
JC = N // 128                # 32 j-chunks
IC = LOCAL // 128            # 4 local i-chunks
FC = F // 128                # 4 contraction chunks
NL = 3
SLOT = 68                    # 64 h' + ec + f2a_hi + f2a_lo + r
CH = H * SLOT                # 544: per-j-chunk stride in GS
HSLOT = 4 * SLOT             # 272: half-phase-A matmul width
ALPHA = 0.2
f32 = mybir.dt.float32
bf16 = mybir.dt.bfloat16
BF = ml_dtypes.bfloat16
OP = mybir.AluOpType
AF = mybir.ActivationFunctionType


def mask_on_g(jc: int) -> bool:
    # ~14/32 mask multiplies per head on gpsimd
    return jc % 5 < 2 or jc == 30


# C-phase head groups: pairs share one wide max op.
CGROUPS = [[0, 1], [2, 3], [4, 5], [6, 7]]


def build_nc():
    nc = bacc.Bacc(None, target_bir_lowering=False, num_devices=NCORES)

    xT_d = nc.dram_tensor("xT", [F, LOCAL], bf16, kind="ExternalInput")
    maskT_d = nc.dram_tensor("maskT", [N, LOCAL], bf16, kind="ExternalInput")
    wext_d = nc.dram_tensor("wext", [NL, F, H * SLOT], bf16, kind="ExternalInput")
    out_d = nc.dram_tensor("outT", [D, LOCAL], f32, kind="ExternalOutput")

    with tile.TileContext(nc) as tc:
        with (
            tc.tile_pool(name="persist", bufs=1) as pp,
            tc.tile_pool(name="ident", bufs=1) as ident_pool,
            tc.tile_pool(name="small", bufs=3) as sm_pool,
            tc.tile_pool(name="hcz", bufs=2) as hcz_pool,
            tc.tile_pool(name="lnr", bufs=4) as lnr_pool,
            tc.tile_pool(name="vr", bufs=6) as vr_pool,
            tc.tile_pool(name="tm", bufs=6) as tm_pool,
            tc.tile_pool(name="pm", bufs=6) as pm_pool,
            tc.tile_pool(name="norm", bufs=2) as nm_pool,
            tc.tile_pool(name="psA", bufs=3, space="PSUM") as psA,
            tc.tile_pool(name="psB", bufs=3, space="PSUM") as psB,
            tc.tile_pool(name="psT", bufs=2, space="PSUM") as psT,
            tc.tile_pool(name="dram", bufs=1, space="DRAM") as dram,
        ):
            # ---- persistent SBUF ----
            XT = pp.tile([128, FC * LOCAL], bf16, tag="XT")        # x^T local
            MASK = pp.tile([128, JC * LOCAL], bf16, tag="MASK")    # mask^T
            WEXT = pp.tile([128, NL * FC * H * SLOT], bf16, tag="WEXT")
            GS = pp.tile([128, JC * CH], bf16, tag="GS")           # gathered stationary
            F1B = pp.tile([128, H * LOCAL], f32, tag="F1B")        # f1 row bcast/head
            EAT = pp.tile([128, H * LOCAL], bf16, tag="EAT")       # exp(f1)
            F12 = pp.tile([128, IC * H * 2], f32, tag="F12")       # f1,f2 stash
            ACCa = pp.tile([D, LOCAL], f32, tag="ACCa")
            ACCb = pp.tile([D, LOCAL], f32, tag="ACCb")
            OUTS = pp.tile([D, LOCAL], f32, tag="OUTS")
            IDENT = ident_pool.tile([128, 128], f32)

            # DRAM bounce buffers
            LGs = [
                dram.tile([LOCAL, SLOT], bf16, tag=f"LG{h}", name=f"LG{h}")
                for h in range(H)
            ]
            GGs = [
                [
                    dram.tile(
                        [N, SLOT], bf16, tag=f"GG{l}_{h}", addr_space="Shared",
                        name=f"GG{l}_{h}",
                    )
                    for h in range(H)
                ]
                for l in range(NL)
            ]
            EDRF = dram.tile([H, LOCAL], f32, tag="EDRF")     # f1 rows bounce
            RBD = dram.tile([1, LOCAL], f32, tag="RBD")       # 1/Z bounce

            # views
            X2 = XT[:].rearrange("p (fc i) -> p fc i", i=LOCAL)
            M2 = MASK[:].rearrange("p (c i) -> p c i", i=LOCAL)
            W4 = WEXT[:].rearrange("p (l fc s) -> p l fc s", l=NL, fc=FC)
            GS3 = GS[:].rearrange("p (h c s) -> p h c s", h=H, s=SLOT)
            GSF = (
                GS[:]
                .bitcast(f32)
                .rearrange("p (h c s) -> p h c s", h=H, s=SLOT // 2)
            )
            F1B2 = F1B[:].rearrange("p (h i) -> p h i", h=H)
            F12v = F12[:].rearrange("p (ic h t) -> p ic h t", h=H, t=2)
            EDRF2 = EDRF[:].rearrange("h (ic c) -> h ic c", c=128)

            # ---- one-time loads ----
            nc.sync.dma_start(X2, xT_d[:].rearrange("(fc p) i -> p fc i", p=128))
            nc.sync.dma_start(M2, maskT_d[:].rearrange("(c p) i -> p c i", p=128))
            nc.sync.dma_start(
                W4, wext_d[:].rearrange("l (fc p) s -> p l fc s", p=128)
            )
            masks.make_identity(nc, IDENT[:])

            for l in range(NL):
                # ---- Phase A: hext = x @ Wext, 4 heads per matmul ----
                for half in range(2):
                    hvs = []
                    hvfs = []
                    for hh in range(4):
                        hcz = hcz_pool.tile(
                            [128, IC * SLOT], bf16, tag=f"hcz{hh}",
                            name=f"hcz{hh}",
                        )
                        hvs.append(
                            hcz[:].rearrange("p (ic s) -> p ic s", s=SLOT)
                        )
                        hvfs.append(
                            hcz[:]
                            .bitcast(f32)
                            .rearrange("p (ic s) -> p ic s", s=SLOT // 2)
                        )
                    for ic in range(IC):
                        ps = psA.tile([128, HSLOT], f32, tag="psA")
                        for fc in range(FC):
                            nc.tensor.matmul(
                                ps[:],
                                X2[:, fc, ic * 128 : (ic + 1) * 128],
                                W4[:, l, fc, half * HSLOT : (half + 1) * HSLOT],
                                start=(fc == 0),
                                stop=(fc == FC - 1),
                            )
                        psv = ps[:].rearrange("p (hh s) -> p hh s", s=SLOT)
                        # stash f1,f2 + ec-scaled h' per head chunk
                        ec4 = sm_pool.tile([128, 4], f32, tag="ec4")
                        nc.scalar.activation(ec4[:], psv[:, :, D + 1], AF.Exp)
                        for hh in range(4):
                            h = half * 4 + hh
                            nc.vector.tensor_copy(
                                F12v[:, ic, h, :], psv[:, hh, D : D + 2]
                            )
                            nc.scalar.activation(
                                hvs[hh][:, ic, 0:D], psv[:, hh, 0:D],
                                AF.Copy, scale=ec4[:, hh : hh + 1],
                            )
                    # per-head tail for this half: gather cols + A2 + collective
                    for hh in range(4):
                        h = half * 4 + hh
                        hv = hvs[hh]
                        # ec column (bf16)
                        nc.scalar.activation(
                            hv[:, :, D], F12v[:, :, h, 1], AF.Exp
                        )
                        # f2a = (a-1)*f2 carried as raw f32 bytes in slot
                        # cols 66:68 (4B-aligned); pad col 65 zeroed.
                        nc.vector.memset(hv[:, :, D + 1], 0.0)
                        nc.vector.tensor_scalar(
                            hvfs[hh][:, :, SLOT // 2 - 1], F12v[:, :, h, 1],
                            ALPHA - 1.0, None, OP.mult,
                        )
                        nc.scalar.dma_start(
                            LGs[h][:].rearrange("(ic p) s -> p ic s", p=128),
                            hv[:, :, :],
                        )
                        # A2: f1 row -> broadcast f1 (f32), exp rows (bf16)
                        pt = psT.tile([IC, 128], f32, tag="psT")
                        nc.tensor.transpose(pt[:], F12v[:, :, h, 0], IDENT[:])
                        f1r = sm_pool.tile([IC, 128], f32, tag="f1r")
                        nc.vector.tensor_copy(f1r[:], pt[:])
                        nc.sync.dma_start(EDRF2[h], f1r[:])
                        nc.sync.dma_start(
                            F1B2[:, h, :],
                            EDRF[h : h + 1, :].partition_broadcast(128),
                        )
                        # exp(f1) broadcast derived on-chip from F1B
                        nc.scalar.activation(
                            EAT[:, h * LOCAL : (h + 1) * LOCAL],
                            F1B2[:, h, :], AF.Exp,
                        )
                        nc.gpsimd.collective_compute(
                            "AllGather",
                            OP.bypass,
                            replica_groups=[list(range(NCORES))],
                            ins=[LGs[h].opt()],
                            outs=[GGs[l][h].opt()],
                        )

                # ---- Phase B: prefetch all staging (sync queue) ----
                for h in range(H):
                    gsrc = GGs[l][h][:].rearrange("(c p) s -> p c s", p=128)
                    nc.sync.dma_start(GS3[:, h, :, :], gsrc[:, :, :])

                # ---- Phase C/D per head group ----
                for grp in CGROUPS:
                    ng = len(grp)
                    h0 = grp[0]
                    pbs = []
                    for k in range(ng):
                        pb = psB.tile(
                            [SLOT - 3, LOCAL], f32, tag="psB", name=f"pb{k}"
                        )
                        pbs.append(pb)
                    for jc in range(JC):
                        vr2 = vr_pool.tile(
                            [128, ng * LOCAL], bf16, tag=f"vr2_{ng}"
                        )
                        for k, h in enumerate(grp):
                            dst = vr2[:, k * LOCAL : (k + 1) * LOCAL]
                            nc.scalar.activation(
                                dst, F1B2[:, h, :], AF.Exp,
                                bias=GSF[:, h, jc, SLOT // 2 - 1 : SLOT // 2],
                                scale=ALPHA,
                            )
                        tm2 = tm_pool.tile(
                            [128, ng * LOCAL], bf16, tag=f"tm2_{ng}"
                        )
                        nc.vector.tensor_tensor(
                            tm2[:], vr2[:],
                            EAT[:, h0 * LOCAL : (h0 + ng) * LOCAL], OP.max,
                        )
                        for k, h in enumerate(grp):
                            pm = pm_pool.tile([128, LOCAL], bf16, tag="pm")
                            eng = nc.gpsimd if mask_on_g(jc) else nc.vector
                            eng.tensor_tensor(
                                pm[:], tm2[:, k * LOCAL : (k + 1) * LOCAL],
                                M2[:, jc, :], OP.mult,
                            )
                            nc.tensor.matmul(
                                pbs[k][:],
                                GS3[:, h, jc, 0 : D + 1],
                                pm[:],
                                start=(jc == 0),
                                stop=(jc == JC - 1),
                            )

                    # ---- Phase D per head: h' = U/Z, ELU ----
                    for k, h in enumerate(grp):
                        pb = pbs[k]
                        zrow1 = nm_pool.tile([1, LOCAL], f32, tag="zrow1")
                        nc.vector.tensor_copy(zrow1[:], pb[D : D + 1, :])
                        zz4 = nm_pool.tile([128, LOCAL // 128], f32, tag="zz4")
                        nc.sync.dma_start(zz4[:], zrow1[:])
                        rz4 = nm_pool.tile([128, LOCAL // 128], f32, tag="rz4")
                        nc.vector.reciprocal(rz4[:], zz4[:])
                        nc.sync.dma_start(RBD[:], rz4[:])
                        rb = nm_pool.tile([D, LOCAL], f32, tag="rb")
                        nc.sync.dma_start(
                            rb[:], RBD[0:1, :].partition_broadcast(D)
                        )
                        y = nm_pool.tile([D, LOCAL], f32, tag="y")
                        nc.vector.tensor_tensor(y[:], pb[0:D, :], rb[:], OP.mult)
                        ee = nm_pool.tile([D, LOCAL], f32, tag="ee")
                        nc.scalar.activation(ee[:], y[:], AF.Exp)
                        ry = nm_pool.tile([D, LOCAL], f32, tag="ry")
                        nc.scalar.activation(ry[:], y[:], AF.Relu)
                        z1 = nm_pool.tile([D, LOCAL], f32, tag="z1")
                        nc.vector.scalar_tensor_tensor(
                            z1[:], ee[:], 1.0, ry[:], OP.min, OP.add
                        )
                        if l < NL - 1:
                            poff = (h % 2) * D
                            dst = X2[poff : poff + D, h // 2, :]
                            nc.vector.tensor_scalar(
                                dst, z1[:], 1.0, None, OP.subtract
                            )
                        else:
                            ey = nm_pool.tile([D, LOCAL], f32, tag="ey")
                            nc.vector.tensor_scalar(
                                ey[:], z1[:], 1.0, None, OP.subtract
                            )
                            if h == 0:
                                nc.vector.tensor_copy(ACCa[:], ey[:])
                            else:
                                src, dst_acc = (
                                    (ACCa, ACCb) if h % 2 == 1 else (ACCb, ACCa)
                                )
                                nc.vector.tensor_tensor(
                                    dst_acc[:], src[:], ey[:], OP.add
                                )

            # ---- final: mean over heads, ELU, write out ----
            fin = ACCb if (H - 1) % 2 == 1 else ACCa
            m1 = nm_pool.tile([D, LOCAL], f32, tag="m1")
            nc.vector.tensor_scalar(m1[:], fin[:], 1.0 / H, None, OP.mult)
            e2 = nm_pool.tile([D, LOCAL], f32, tag="e2")
            nc.scalar.activation(e2[:], m1[:], AF.Exp)
            r2 = nm_pool.tile([D, LOCAL], f32, tag="r2")
            nc.scalar.activation(r2[:], m1[:], AF.Relu)
            nc.vector.scalar_tensor_tensor(
                OUTS[:], e2[:], 1.0, r2[:], OP.min, OP.add
            )
            nc.vector.tensor_scalar(OUTS[:], OUTS[:], 1.0, None, OP.subtract)
            nc.sync.dma_start(out_d[:], OUTS[:])

    nc.compile()
    return nc


def _prep_inputs(inputs):
    x = np.asarray(inputs["x"], np.float32)
    adj = np.asarray(inputs["adj"])
    Ws = [np.asarray(inputs[k], np.float32) for k in ("W1", "W2", "W3")]
    a1s = [np.asarray(inputs[k], np.float32) for k in ("a1_1", "a1_2", "a1_3")]
    a2s = [np.asarray(inputs[k], np.float32) for k in ("a2_1", "a2_2", "a2_3")]

    wext = np.zeros((NL, F, H * SLOT), np.float32)
    for l in range(NL):
        for h in range(H):
            wext[l, :, h * SLOT : h * SLOT + D] = Ws[l][h]
            wext[l, :, h * SLOT + D] = Ws[l][h] @ a1s[l][h]
            wext[l, :, h * SLOT + D + 1] = Ws[l][h] @ a2s[l][h]
    wext_bf = np.ascontiguousarray(wext.astype(BF))

    mask = adj > 0
    in_maps = []
    for c in range(NCORES):
        rows = slice(c * LOCAL, (c + 1) * LOCAL)
        in_maps.append(
            {
                "xT": np.ascontiguousarray(x[rows].T).astype(BF),
                "maskT": np.ascontiguousarray(mask[rows].T).astype(BF),
                "wext": wext_bf,
            }
        )
    return in_maps


_CACHE = {}


def _run(inputs, trace=False):
    in_maps = _prep_inputs(inputs)
    if "nc" not in _CACHE:
        _CACHE["nc"] = build_nc()
    res = run_bass_kernel_spmd(
        _CACHE["nc"], in_maps, list(range(NCORES)), trace=trace
    )
    outs = [r["outT"] for r in res.results]
    out = np.concatenate([np.asarray(o, np.float32).T for o in outs], axis=0)
    return out, res


def kernel(**inputs) -> np.ndarray:
    out, _ = _run(inputs, trace=False)
    return out.astype(np.float32)
